# revision 7
# baseline (speedup 1.0000x reference)
"""Trainium2 Bass kernel for nn_EnergyFunction (8-core SPMD).

Reference computation (per batch b):
    Q = features @ Wq;  K = features @ Wk                     # [S, 64]
    scores = (Q @ K.T) / 8 * locality_scale / max(|i-j|, 1)   # [S, S]
    charge = sigmoid(features @ w_charge + b_charge)          # [S]
    energy = -scores * charge_i * charge_j

Sharding: core = (b, i-half). Each of the 8 cores handles one batch b
(= core // 2) and one half of the query rows (i0 = (core % 2) * 2048),
producing a [2048, 4096] block of the [4, 4096, 4096] output.

Device-side plan (per core), fp32r matmuls throughout:
  - Load features[b].T ("fK" [512, 4096]) and its query-half slice
    ("fQ" [512, 2048]) in feature-major layout (host pre-transposes).
  - K-side prelim: lhsT = [Wk | w_charge] -> psum [65, 512] segs:
    rows 0:64 = K^T, row 64 = charge logits. Sigmoid (ACT) -> crow;
    broadcast crow to 64 partitions via ones[1,64] matmul; fold
    K'^T = K^T * c_j (DVE, writes fp32r).
  - Q-side prelim: lhsT = [Wq * (-loc/8) | w_charge] over fQ, same
    pipeline -> Q'^T = Q^T * c_i.
  - Main loop (16 i-tiles x 4 j-pairs): 2x PE matmul [64c,128m,512n]
    into a 2-bank psum [128, 1024] -> DVE tensor_mul with the Toeplitz
    mask band slice -> 512 KB DMA out.
    Mask band: vb2d[p, u] = 1/max(|i_base + 1920 + p - u|, 1)
    (host input [128, 6016]; tile (t, j) uses u0 = 1024 j - 128 t + 1920).
"""

import numpy as np

import concourse.bass as bass
import concourse.bacc as bacc
import concourse.mybir as mybir
from concourse import tile
from concourse import bass_utils

# Problem shape (hardcoded per harness contract)
B = 4
S = 4096
F = 512
D = 64

P = 128            # partition tile (i)
SEG = 512          # j segment width (one PSUM bank of fp32)
WOUT = 2048        # epilogue / output tile width (4 PSUM banks)
IHALF = S // 2     # 2048 query rows per core
NIT = IHALF // P   # 16 i-tiles
NSEG = S // SEG    # 8 j segments
NJP = S // WOUT    # 4 j output tiles per i-tile
NQSEG = IHALF // SEG  # 4 q segments
NCH = F // P       # 4 feature chunks
C0 = IHALF - P     # 1920 mask-band column offset
MBW = (S - SEG) + C0 + SEG  # 6016 mask band width

F32 = mybir.dt.float32
F32R = mybir.dt.float32r
SIG = mybir.ActivationFunctionType.Sigmoid
COPY = mybir.ActivationFunctionType.Copy

_PROGRAM = None


def _build_program():
    nc = bacc.Bacc("TRN2", target_bir_lowering=False, debug=False, num_devices=8)

    fK = nc.dram_tensor("fK", [F, S], F32R, kind="ExternalInput").ap()
    # [Wk | w_charge] and [Wq * (-loc/8) | w_charge], both [F, 65]
    wk65 = nc.dram_tensor("wk65", [F, D + 1], F32R, kind="ExternalInput").ap()
    wq65 = nc.dram_tensor("wq65", [F, D + 1], F32R, kind="ExternalInput").ap()
    bvec = nc.dram_tensor("bvec", [P, 1], F32, kind="ExternalInput").ap()
    ones = nc.dram_tensor("ones", [1, D], F32, kind="ExternalInput").ap()
    vb2d = nc.dram_tensor("vb2d", [P, MBW], F32, kind="ExternalInput").ap()
    energy = nc.dram_tensor("energy", [IHALF, S], F32, kind="ExternalOutput").ap()

    W65 = D + 1

    with tile.TileContext(nc) as tc:
        with (
            tc.tile_pool(name="const", bufs=1) as const,
            tc.tile_pool(name="stage", bufs=1) as stage,
        ):
            bvec_sb = const.tile([P, 1], F32, tag="bvec")
            nc.sync.dma_start(out=bvec_sb[:], in_=bvec)
            ones_sb = const.tile([1, D], F32, tag="ones")
            nc.sync.dma_start(out=ones_sb[:], in_=ones)
            wk_sb = const.tile([P, NCH * W65], F32R, tag="wk")
            wq_sb = const.tile([P, NCH * W65], F32R, tag="wq")
            for c in range(NCH):
                nc.sync.dma_start(
                    out=wk_sb[:, c * W65:(c + 1) * W65],
                    in_=wk65[c * P:(c + 1) * P, :],
                )
                nc.sync.dma_start(
                    out=wq_sb[:, c * W65:(c + 1) * W65],
                    in_=wq65[c * P:(c + 1) * P, :],
                )

            # Persistent prelim outputs
            QT = stage.tile([D, IHALF], F32R, tag="qt")    # Q^T * c_i
            KpT = stage.tile([D, S], F32R, tag="kpt")      # K^T * c_j
            crow = stage.tile([1, S], F32, tag="crow")     # K-side charge row
            qrow = stage.tile([1, IHALF], F32, tag="qrow")  # Q-side charge row
            vb_sb = stage.tile([P, MBW], F32, tag="vb")

            with (
                tc.tile_pool(name="feat", bufs=1) as fpool,
                tc.tile_pool(name="pk", space="PSUM", bufs=2) as ps_k,
                tc.tile_pool(name="pq", space="PSUM", bufs=2) as ps_q,
                tc.tile_pool(name="pC", space="PSUM", bufs=2) as ps_C,
            ):
                fk = []
                for c in range(NCH):
                    t = fpool.tile([P, S], F32R, tag=f"fk{c}")
                    nc.sync.dma_start(out=t[:], in_=fK[c * P:(c + 1) * P, :])
                    fk.append(t)
                # mask band load is only needed by the main loop; emit late
                nc.sync.dma_start(out=vb_sb[:], in_=vb2d)

                # query-half offset into fK, from the SPMD partition id
                pid = nc.tensor.partition_id()
                qoff = (pid % 2) * IHALF

                # Software-pipelined projection chains. Stages per seg:
                #   4 accumulating matmuls -> ACT sigmoid (charge row) ->
                #   ones-matmul broadcast -> ACT copy -> DVE fold multiply.
                # The broadcast matmul for seg s-1 is emitted after seg s's
                # matmul group so the PE never waits on the ACT sigmoid.
                segs = [("k", s) for s in range(4)] + [("q", 0)]
                segs += [("k", s) for s in range(4, NSEG)]
                segs += [("q", s) for s in range(1, NQSEG)]
                pending = None  # (pXX, side, s) awaiting broadcast+fold

                def _emit_bcast_fold(pX, side, s):
                    row = crow if side == "k" else qrow
                    dst = KpT if side == "k" else QT
                    pC = ps_C.tile([D, SEG], F32, tag="pC")
                    nc.tensor.matmul(
                        pC[:], ones_sb[:], row[0:1, s * SEG:(s + 1) * SEG],
                        start=True, stop=True,
                    )
                    Cb = stage.tile([D, SEG], F32, tag="cb", bufs=2)
                    nc.scalar.activation(Cb[:], pC[:], COPY)
                    nc.vector.tensor_mul(
                        out=dst[:, s * SEG:(s + 1) * SEG],
                        in0=pX[0:D, :],
                        in1=Cb[:],
                    )

                for side, s in segs:
                    w_sb = wk_sb if side == "k" else wq_sb
                    row = crow if side == "k" else qrow
                    pool = ps_k if side == "k" else ps_q
                    pX = pool.tile([W65, SEG], F32, tag="pk" if side == "k" else "pq")
                    for c in range(NCH):
                        if side == "k":
                            rhs = fk[c][:, s * SEG:(s + 1) * SEG]
                        else:
                            rhs = fk[c][:, bass.ds(qoff + s * SEG, SEG)]
                        nc.tensor.matmul(
                            pX[:],
                            w_sb[:, c * W65:(c + 1) * W65],
                            rhs,
                            start=(c == 0),
                            stop=(c == NCH - 1),
                        )
                    nc.scalar.activation(
                        row[0:1, s * SEG:(s + 1) * SEG], pX[D:D + 1, :],
                        SIG, bias=bvec_sb[0:1, :], scale=1.0,
                    )
                    if pending is not None:
                        _emit_bcast_fold(*pending)
                    pending = (pX, side, s)
                if pending is not None:
                    _emit_bcast_fold(*pending)

            # ---- main loop ----
            NSH = WOUT // SEG  # matmuls per output tile
            with (
                tc.tile_pool(name="pse", space="PSUM", bufs=2) as ps_e,
                tc.tile_pool(name="osb", bufs=3) as opool,
            ):
                for t in range(NIT):
                    for j in range(NJP):
                        pe_ = ps_e.tile([P, WOUT], F32)
                        for h in range(NSH):
                            nc.tensor.matmul(
                                pe_[:, h * SEG:(h + 1) * SEG],
                                QT[:, t * P:(t + 1) * P],
                                KpT[:, (NSH * j + h) * SEG:(NSH * j + h + 1) * SEG],
                                start=True,
                                stop=True,
                            )
                        osb = opool.tile([P, WOUT], F32)
                        u0 = j * WOUT - t * P + C0
                        nc.vector.tensor_mul(
                            out=osb[:],
                            in0=pe_[:],
                            in1=vb_sb[:, u0:u0 + WOUT],
                        )
                        nc.sync.dma_start(
                            out=energy[t * P:(t + 1) * P, j * WOUT:(j + 1) * WOUT],
                            in_=osb[:],
                        )

    nc.compile()
    return nc


def _get_program():
    global _PROGRAM
    if _PROGRAM is None:
        _PROGRAM = _build_program()
    return _PROGRAM


def _make_in_maps(features, Wq, Wk, w_charge, b_charge, loc):
    wq_s = Wq * np.float32(-loc / 8.0)
    wq65 = np.ascontiguousarray(
        np.concatenate([wq_s, w_charge[:, None]], axis=1)
    )
    wk65 = np.ascontiguousarray(
        np.concatenate([Wk, w_charge[:, None]], axis=1)
    )
    bvec = np.full((P, 1), b_charge, dtype=np.float32)
    ones = np.ones((1, D), dtype=np.float32)

    u = np.arange(MBW, dtype=np.float32)[None, :]
    vb_half = []
    for h in range(2):
        ib = (h * IHALF + C0 + np.arange(P, dtype=np.float32))[:, None]
        vb_half.append(
            np.ascontiguousarray(1.0 / np.maximum(np.abs(ib - u), 1.0))
        )

    fT = [np.ascontiguousarray(features[b].T) for b in range(B)]

    in_maps = []
    for core in range(2 * B):
        b, h = divmod(core, 2)
        in_maps.append({
            "fK": fT[b],
            "wk65": wk65,
            "wq65": wq65,
            "bvec": bvec,
            "ones": ones,
            "vb2d": vb_half[h],
        })
    return in_maps


def kernel(features, Wq, Wk, w_charge, b_charge, locality_scale):
    features = np.asarray(features, dtype=np.float32)
    Wq = np.asarray(Wq, dtype=np.float32)
    Wk = np.asarray(Wk, dtype=np.float32)
    w_charge = np.asarray(w_charge, dtype=np.float32)
    b_charge = float(np.asarray(b_charge))
    loc = float(np.asarray(locality_scale))

    nc = _get_program()
    in_maps = _make_in_maps(features, Wq, Wk, w_charge, b_charge, loc)
    res = bass_utils.run_bass_kernel_spmd(nc, in_maps, core_ids=list(range(2 * B)))

    out = np.empty((B, S, S), dtype=np.float32)
    for core in range(2 * B):
        b, h = divmod(core, 2)
        out[b, h * IHALF:(h + 1) * IHALF, :] = res.results[core]["energy"]
    return out
